# revision 25
# baseline (speedup 1.0000x reference)
"""MultiHeadedAttention Trainium2 kernel (8-core SPMD, data-parallel).

Sharding: 8 cores = (batch b in 0..3) x (query half in 0..1). Each core
computes out[b, half*1024:(half+1)*1024, :] independently - no collectives.

Per-core dataflow v2 (no DRAM staging; transposes on the PE):
  - x blocks [128, dm] fp32 DMA'd natural, cast fp32->bf16 on DVE,
    transposed 128x128 at a time on the PE (identity matmul) into bf16
    PSUM, evicted by ScalarE into xT layouts (contraction dim on
    partitions)
  - v: per-block xvT ring -> v natural [Sk, d] + ones column (Z trick)
  - mask: int32 blocks cast to bf16, PE-transposed, GpSimd-evicted to
    maskT
  - k/q: full xkT/xqT, projections tiled ns-outer/hp-inner so the
    PE consumes transposed blocks as they land; bias applied during
    PSUM eviction on DVE (tensor_scalar_add)
  - attention per head-pair: scores^T = kT.T @ qT (row-tiled head
    pairs); exp on ScalarE (scale 1/8 folded into Wq/bq); mask applied
    as bf16 multiply on DVE; PV psum rows 0..63 = sum_j v^T p, row 64
    = Z; finalize: reciprocal on DVE, partition-broadcast on GpSimd,
    multiply, SBUF->SBUF DMA hop into head-pair layout xattnT
  - out = xattnT.T @ WoT + R where R = bo + bv@WoT (PE-broadcast), fp32
"""
import numpy as np
import ml_dtypes

import concourse.bass as bass
import concourse.mybir as mybir
import concourse.tile as tile
from concourse import bacc
from concourse.bass_utils import run_bass_kernel_spmd
from concourse.masks import make_identity

F32 = mybir.dt.float32
BF16 = mybir.dt.bfloat16
I32 = mybir.dt.int32
AF = mybir.ActivationFunctionType
ALU = mybir.AluOpType

N_CORES = 8
DK = 64


def slices(total, chunk):
    return [(s, min(chunk, total - s)) for s in range(0, total, chunk)]


class Cfg:
    def __init__(self, SQ=1024, SK=2048, DM=1024, H=16, max_stage=5):
        assert DM % 128 == 0 and SK % 128 == 0 and SQ % 128 == 0 and H % 2 == 0
        self.SQ, self.SK, self.DM, self.H = SQ, SK, DM, H
        self.KT = DM // 128          # dm contraction chunks
        self.HP = H // 2             # head pairs
        self.NJ = SK // 128          # Sk tiles
        self.SQS = min(1024, SQ)     # attention Sq slice width (2 psum banks)
        self.max_stage = max_stage   # debug: truncate kernel after stage N
        assert SQ % self.SQS == 0
        assert H * DK == DM


def emit_kernel(tc, cfg, io):
    nc = tc.nc
    C = cfg
    xq, xk, xv, msk = io["xq"], io["xk"], io["xv"], io["mask"]
    w_dram = {"q": io["wqt"], "k": io["wkt"], "v": io["wvt"], "o": io["wot"]}
    bql, bkl, rfull = io["bql"], io["bkl"], io["rfull"]
    out = io["out"]

    pools = {}

    def open_pool(name, bufs=1, space="SBUF", side=None):
        pools[name] = tc.alloc_tile_pool(name=name, bufs=bufs, space=space,
                                         side=side)
        return pools[name]

    def release_pool(name):
        pools[name].release()
        del pools[name]

    persist = open_pool("persist", 1)
    # Stage A-C PSUM: proj psums (tag s, 2x2 banks) + transpose staging
    # (tag t, 2x1 bank). Released before attention, which re-opens the
    # arena as scores (2x2) + PV accumulators (2x2).
    ps_s = open_pool("ps_s", 2, space="PSUM")
    ps_t = open_pool("ps_t", 3, space="PSUM")
    # wo on the right-side SBUF stack: loaded late, lives to the end
    wo_pool = open_pool("wo", 1, side="right")
    ring = open_pool("ring", 1)      # xn/xb: lives through stages A-C
    ringA = open_pool("ringA", 1)    # xvt/mi/mb: stage A only

    # ---------------- persistent tiles ----------------
    qT_sb = persist.tile([128, C.HP * C.SQ], BF16, name="qT_sb")
    kT_sb = persist.tile([128, C.HP * C.SK], BF16, name="kT_sb")
    v_sb = persist.tile([128, C.NJ * C.H * 65], BF16, name="v_sb")
    xattnT_sb = persist.tile([128, C.HP * C.SQ], BF16, name="xattnT_sb")
    maskT_sb = persist.tile([128, C.NJ * C.SQ], BF16, name="maskT_sb")
    R_sb = persist.tile([128, C.DM], F32, name="R_sb")
    bql_sb = persist.tile([128, C.HP], F32, name="bql_sb")
    bkl_sb = persist.tile([128, C.HP], F32, name="bkl_sb")
    ident_sb = persist.tile([128, 128], BF16, name="ident_sb")

    make_identity(nc, ident_sb[:])
    nc.sync.dma_start(bql_sb[:], bql[:])
    nc.sync.dma_start(bkl_sb[:], bkl[:])

    PS_F = max(C.SQS, 512)

    maskTv = maskT_sb.rearrange("p (j s) -> p j s", j=C.NJ)

    def load_w(pool, which, name):
        w_sb = pool.tile([128, C.KT * C.DM], BF16, name=name)
        for kt in range(C.KT):
            nc.sync.dma_start(w_sb[:, kt * C.DM:(kt + 1) * C.DM],
                               w_dram[which][kt * 128:(kt + 1) * 128, :])
        return w_sb

    def load_x_block(x_in, r):
        """DMA 128-row block r of x (fp32) and cast to bf16; returns the
        bf16 tile (natural layout)."""
        xn = ring.tile([128, C.DM], F32, name="xn", tag="xn", bufs=3)
        nc.sync.dma_start(xn[:], x_in[r * 128:(r + 1) * 128, :])
        xb = ring.tile([128, C.DM], BF16, name="xb", tag="xb", bufs=5)
        nc.vector.tensor_copy(xb[:], xn[:])
        return xb

    def tev_block(xb, dst):
        """PE-transpose a bf16 natural block into dst [p, kt, 128]."""
        pt = ps_t.tile([128, C.KT * 128], BF16, name="pt", tag="t",
                       padded_shape=[128, 1024])
        for kt in range(C.KT):
            nc.tensor.transpose(pt[:, kt * 128:(kt + 1) * 128],
                                xb[:, kt * 128:(kt + 1) * 128], ident_sb[:])
        nc.scalar.activation(
            dst,
            pt.rearrange("p (kt s) -> p kt s", kt=C.KT),
            AF.Copy,
        )

    def emit_x_block(x_in, r, dst):
        tev_block(load_x_block(x_in, r), dst)

    def emit_mask_block(r):
        """Load 128-row block r of mask (int32), cast to bf16, PE-transpose,
        GpSimd-evict into maskT."""
        for half in range(C.SK // 1024):
            mi = ringA.tile([128, 1024], I32, name="mi", tag="mi", bufs=2)
            nc.gpsimd.dma_start(
                mi[:], msk[r * 128:(r + 1) * 128,
                           half * 1024:(half + 1) * 1024])
            mb = ringA.tile([128, 1024], BF16, name="mb", tag="mb", bufs=2)
            nc.gpsimd.tensor_copy(mb[:], mi[:])
            pt = ps_t.tile([128, 1024], BF16, name="ptm", tag="t",
                           padded_shape=[128, 1024])
            for c in range(8):
                nc.tensor.transpose(pt[:, c * 128:(c + 1) * 128],
                                    mb[:, c * 128:(c + 1) * 128], ident_sb[:])
            nc.scalar.activation(
                maskTv[:, half * 8:(half + 1) * 8, r * 128:(r + 1) * 128],
                pt.rearrange("p (c s) -> p c s", c=8),
                AF.Copy,
            )

    # ---------------- stage A: v (ring) + mask, interleaved ----------------
    wv_pool = open_pool("wv", 1)
    wv_sb = load_w(wv_pool, "v", "w_v")
    v_view = v_sb.rearrange("p (j h c) -> p j h c", j=C.NJ, c=65)

    NBV = C.SK // 128
    xvt_blocks = [None] * NBV

    def emit_v_block(r):
        xvt = ringA.tile([128, C.KT * 128], BF16, name="xvt", tag="xvt", bufs=3)
        xvt_blocks[r] = xvt
        emit_x_block(xv, r, xvt.rearrange("p (kt s) -> p kt s", kt=C.KT)[:, :, :])

    def emit_v_proj(j):
        xvt = xvt_blocks[j]
        for (ds_, dw) in slices(C.DM, 512):
            hs = ds_ // DK
            ps = ps_s.tile([128, dw], F32, name="ps_v", tag="s",
                           padded_shape=[128, PS_F])
            for kt in range(C.KT):
                nc.tensor.matmul(
                    ps[:],
                    xvt[:, kt * 128:(kt + 1) * 128],
                    wv_sb[:, kt * C.DM + ds_: kt * C.DM + ds_ + dw],
                    start=(kt == 0), stop=(kt == C.KT - 1),
                )
            nc.vector.tensor_copy(
                v_view[:, j, hs:hs + dw // DK, 0:64],
                ps.rearrange("p (h c) -> p h c", c=DK),
            )
        xvt_blocks[j] = None

    # interleave: 3-block lead for the transpose pipeline; one mask block
    # every other v block
    LEAD = 3
    n_mask = C.SQ // 128
    mq = list(range(n_mask))
    for r in range(LEAD):
        emit_v_block(r)
    pre_k = None
    for r in range(NBV):
        if r + LEAD < NBV:
            emit_v_block(r + LEAD)
        if r == NBV - 2:
            pre_k = [load_x_block(xk, rr) for rr in range(4)]
        emit_v_proj(r)
        if r >= 2 and mq:
            emit_mask_block(mq.pop(0))
    for r in mq:
        emit_mask_block(r)
    nc.vector.memset(v_view[:, :, :, 64:65], 1.0)
    release_pool("wv")
    release_pool("ringA")

    if C.max_stage <= 2:
        for pl in reversed(list(pools.values())):
            pl.release()
        return

    # ---------------- stage B/C: k then q projections ----------------
    # ns-outer / hp-inner: output columns [ns, ns+512) need only the four
    # x row-blocks ns/128.., held in a 2-deep ring of transposed groups.
    def proj_stage(pool, x_in, w_sb, T_sb, S, bias_sb, gtag, pre_xbs=None,
                   preload_next=None):
        chunks = slices(S, 512)
        preloaded = None

        def emit_group(ci, xbs=None):
            grp = pool.tile([128, C.KT * 512], BF16, name=f"{gtag}{ci}",
                            tag=gtag, bufs=2)
            gv = grp.rearrange("p (kt s) -> p kt s", kt=C.KT)
            for rr in range(4):
                xb = xbs[rr] if xbs else load_x_block(x_in, 4 * ci + rr)
                tev_block(xb, gv[:, :, rr * 128:(rr + 1) * 128])
            return grp

        nxt = emit_group(0, pre_xbs)
        for ci, (ns, nw) in enumerate(chunks):
            grp, nxt = nxt, (emit_group(ci + 1) if ci + 1 < len(chunks) else None)
            if ci == len(chunks) - 1 and preload_next is not None:
                preloaded = preload_next()
            for hp in range(C.HP):
                ps = ps_s.tile([128, nw], F32, name="ps_p", tag="s",
                               padded_shape=[128, PS_F])
                for kt in range(C.KT):
                    nc.tensor.matmul(
                        ps[:],
                        w_sb[:, kt * C.DM + hp * 128: kt * C.DM + (hp + 1) * 128],
                        grp[:, kt * 512: kt * 512 + nw],
                        start=(kt == 0), stop=(kt == C.KT - 1),
                    )
                nc.vector.tensor_scalar_add(
                    T_sb[:, hp * S + ns: hp * S + ns + nw],
                    ps[:], bias_sb[:, hp:hp + 1],
                )
        return preloaded

    kx = open_pool("kx", 1)
    wk_sb = load_w(kx, "k", "w_k")
    pre_q = proj_stage(kx, xk, wk_sb, kT_sb, C.SK, bkl_sb, "xkg",
                       pre_xbs=pre_k,
                       preload_next=lambda: [load_x_block(xq, r)
                                             for r in range(4)])
    release_pool("kx")

    qx = open_pool("qx", 1)
    wq_sb = load_w(qx, "q", "w_q")
    wo_sb = wo_pool.tile([128, C.KT * C.DM], BF16, name="wo_sb")
    for kt in range(C.KT):
        nc.sync.dma_start(wo_sb[:, kt * C.DM:(kt + 1) * C.DM],
                          w_dram["o"][kt * 128:(kt + 1) * 128, :])
    proj_stage(qx, xq, wq_sb, qT_sb, C.SQ, bql_sb, "xqg", pre_xbs=pre_q)

    release_pool("qx")
    release_pool("ring")
    release_pool("ps_t")
    release_pool("ps_s")

    if C.max_stage <= 3:
        for pl in reversed(list(pools.values())):
            pl.release()
        return

    # ---------------- attention ----------------
    nc.sync.dma_start(R_sb[:], rfull[:])
    ps_sc = open_pool("ps_sc", 2, space="PSUM")
    ps_pv = open_pool("ps_pv", 2, space="PSUM")
    attn = open_pool("attn", 1)

    # One continuous software-pipelined stream over (hp, j): the PV matmuls
    # lag the scores/exp/mask chain by PIPE items, and the lag crosses
    # head-pair boundaries, so the drain of head-pair hp overlaps the
    # scores/exp of head-pair hp+1 and ScalarE never idles at boundaries.
    sq, sw = 0, C.SQS
    assert C.SQS == C.SQ
    PIPE = 3
    pv_of = {}
    pm_hist = []

    def emit_pv(hp, jj, pms):
        if jj == 0:
            pv_of[hp] = [
                ps_pv.tile([65, sw], F32, name=f"ps_pv{i}", tag="pv",
                           padded_shape=[128, PS_F])
                for i in range(2)
            ]
        pv = pv_of[hp]
        for i in range(2):
            for (qs, qw) in slices(sw, 512):
                nc.tensor.matmul(
                    pv[i][:, qs:qs + qw], v_view[:, jj, 2 * hp + i, :],
                    pms[i][:, qs:qs + qw],
                    start=(jj == 0), stop=(jj == C.NJ - 1),
                )

    def emit_finalize(hp):
        pv = pv_of.pop(hp)
        for i in range(2):
            # Z row to SBUF (the custom DVE reciprocal cannot source PSUM
            # on hardware), reciprocal on DVE, partition-broadcast on
            # GpSimd, normalize on DVE straight from the PSUM accumulator
            zrow = attn.tile([1, sw], F32, name="zrow", tag="zrow", bufs=2,
                             padded_shape=[1, C.SQS])
            nc.vector.tensor_copy(zrow[:], pv[i][64:65, :])
            zr1 = attn.tile([1, sw], F32, name="zr1", tag="zr1", bufs=2,
                            padded_shape=[1, C.SQS])
            nc.vector.reciprocal_approx_fast(out=zr1[:], in_=zrow[:])
            zrb = attn.tile([64, sw], F32, name="zrb", tag="zrb", bufs=2,
                            padded_shape=[64, C.SQS])
            nc.gpsimd.partition_broadcast(zrb[:], zr1[:])
            tmp = attn.tile([64, sw], BF16, name="xat_t", tag="xat_t",
                            bufs=2, padded_shape=[64, C.SQS])
            nc.vector.tensor_tensor(out=tmp[:], in0=pv[i][0:64, :],
                                    in1=zrb[:], op=ALU.mult)
            # partition hop: rows 0..63 -> xattnT pair rows 64i..64i+64
            nc.sync.dma_start(
                xattnT_sb[64 * i:64 * (i + 1),
                          hp * C.SQ + sq: hp * C.SQ + sq + sw],
                tmp[:],
            )

    def drain_one():
        (dhp, jj), pp = pm_hist.pop(0)
        emit_pv(dhp, jj, pp)
        if jj == C.NJ - 1:
            emit_finalize(dhp)

    for hp in range(C.HP):
        for j in range(C.NJ):
            # emit the pipelined PV first so the PE has ready work queued
            # ahead of the ss-slot wait of this iteration's scores
            if len(pm_hist) >= PIPE:
                drain_one()
            pms = []
            sss = [ps_sc.tile([128, sw], F32, name=f"ps_sc{i}", tag="sc",
                              padded_shape=[128, PS_F]) for i in range(2)]
            # interleave the two heads' MMs so the row-tiled (0,0)/(64,0)
            # pairs sit adjacent in the PE queue and run concurrently
            for (qs, qw) in slices(sw, 512):
                for i in range(2):
                    nc.tensor.matmul(
                        sss[i][:, qs:qs + qw],
                        kT_sb[i * 64:(i + 1) * 64,
                              hp * C.SK + j * 128: hp * C.SK + (j + 1) * 128],
                        qT_sb[i * 64:(i + 1) * 64,
                              hp * C.SQ + sq + qs: hp * C.SQ + sq + qs + qw],
                        start=True, stop=True,
                    )
            for i in range(2):
                pe = attn.tile([128, sw], BF16, name="p_exp", tag="pexp",
                               bufs=3, padded_shape=[128, C.SQS])
                nc.scalar.activation(pe[:], sss[i][:], AF.Exp)
                pm = attn.tile([128, sw], BF16, name="p_msk", tag="pmask",
                               bufs=8, padded_shape=[128, C.SQS])
                nc.vector.tensor_tensor(
                    out=pm[:], in0=pe[:],
                    in1=maskT_sb[:, j * C.SQ + sq: j * C.SQ + sq + sw],
                    op=ALU.mult,
                )
                pms.append(pm)
            pm_hist.append(((hp, j), pms))
    while pm_hist:
        drain_one()

    if C.max_stage <= 4:
        for pl in reversed(list(pools.values())):
            pl.release()
        return

    # ---------------- epilogue: output projection ----------------
    # hp-outer in groups of four row-blocks (reusing the attention PSUM
    # pools: 2 score slots + 2 pv slots = 4 concurrent accumulators), so
    # only the final head-pair's matmuls wait on the attention finalize.
    release_pool("attn")
    epi = open_pool("epi", 1)

    NM = C.SQ // 128
    for (ns, nw) in slices(C.DM, 512):
        for mg in range(0, NM, 4):
            po = [ps_sc.tile([128, nw], F32, name=f"ps_o{m}", tag="sc",
                             padded_shape=[128, PS_F]) for m in range(2)]
            po += [ps_pv.tile([128, nw], F32, name=f"ps_o{m+2}", tag="pv",
                              padded_shape=[128, PS_F]) for m in range(2)]
            for hp in range(C.HP):
                for mi_ in range(4):
                    m = mg + mi_
                    nc.tensor.matmul(
                        po[mi_][:],
                        xattnT_sb[:, hp * C.SQ + m * 128: hp * C.SQ + (m + 1) * 128],
                        wo_sb[:, hp * C.DM + ns: hp * C.DM + ns + nw],
                        start=(hp == 0), stop=(hp == C.HP - 1),
                    )
            for mi_ in range(4):
                m = mg + mi_
                ot = epi.tile([128, nw], F32, name="out_sb", tag="out_sb",
                              bufs=4, padded_shape=[128, 512])
                nc.vector.tensor_tensor(out=ot[:], in0=po[mi_][:],
                                        in1=R_sb[:, ns:ns + nw], op=ALU.add)
                nc.sync.dma_start(out[m * 128:(m + 1) * 128, ns:ns + nw], ot[:])

    for pl in reversed(list(pools.values())):
        pl.release()


def build(cfg, reps=1):
    nc = bacc.Bacc("TRN2", target_bir_lowering=False, debug=False)
    C = cfg
    io = {
        "xq": nc.dram_tensor("xq", [C.SQ, C.DM], F32, kind="ExternalInput").ap(),
        "xk": nc.dram_tensor("xk", [C.SK, C.DM], F32, kind="ExternalInput").ap(),
        "xv": nc.dram_tensor("xv", [C.SK, C.DM], F32, kind="ExternalInput").ap(),
        "mask": nc.dram_tensor("mask", [C.SQ, C.SK], I32, kind="ExternalInput").ap(),
        "wqt": nc.dram_tensor("wqt", [C.DM, C.DM], BF16, kind="ExternalInput").ap(),
        "wkt": nc.dram_tensor("wkt", [C.DM, C.DM], BF16, kind="ExternalInput").ap(),
        "wvt": nc.dram_tensor("wvt", [C.DM, C.DM], BF16, kind="ExternalInput").ap(),
        "wot": nc.dram_tensor("wot", [C.DM, C.DM], BF16, kind="ExternalInput").ap(),
        "bql": nc.dram_tensor("bql", [128, C.HP], F32, kind="ExternalInput").ap(),
        "bkl": nc.dram_tensor("bkl", [128, C.HP], F32, kind="ExternalInput").ap(),
        "rfull": nc.dram_tensor("rfull", [128, C.DM], F32, kind="ExternalInput").ap(),
        "out": nc.dram_tensor("out", [C.SQ, C.DM], F32, kind="ExternalOutput").ap(),
    }
    with tile.TileContext(nc) as tc:
        for _ in range(reps):
            emit_kernel(tc, cfg, io)
    nc.compile()
    return nc


def host_prep(query, key, value, mask, Wq, bq, Wk, bk, Wv, bv, Wo, bo, cfg):
    """Host-side layout prep (weight transpose/cast, per-core slicing)."""
    C = cfg
    bf = ml_dtypes.bfloat16
    wqt = np.ascontiguousarray((Wq.T * 0.125).astype(bf))   # 1/sqrt(dk) folded
    wkt = np.ascontiguousarray(Wk.T.astype(bf))
    wvt = np.ascontiguousarray(Wv.T.astype(bf))
    wot = np.ascontiguousarray(Wo.T.astype(bf))
    bql = np.ascontiguousarray((bq * 0.125).reshape(C.HP, 128).T.astype(np.float32))
    bkl = np.ascontiguousarray(bk.reshape(C.HP, 128).T.astype(np.float32))
    r_row = (bv.astype(np.float32) @ Wo.T.astype(np.float32)
             + bo.astype(np.float32))
    rfull = np.ascontiguousarray(
        np.broadcast_to(r_row, (128, C.DM)).astype(np.float32))
    shared = dict(wqt=wqt, wkt=wkt, wvt=wvt, wot=wot, bql=bql, bkl=bkl,
                  rfull=rfull)
    in_maps = []
    B = query.shape[0]
    halves = query.shape[1] // C.SQ
    for c in range(B * halves):
        b, h = divmod(c, halves)
        m = dict(shared)
        m["xq"] = np.ascontiguousarray(query[b, h * C.SQ:(h + 1) * C.SQ, :])
        m["xk"] = np.ascontiguousarray(key[b])
        m["xv"] = np.ascontiguousarray(value[b])
        m["mask"] = np.ascontiguousarray(mask[b, h * C.SQ:(h + 1) * C.SQ, :])
        in_maps.append(m)
    return in_maps


_CACHED = {}


def get_built():
    if "nc" not in _CACHED:
        _CACHED["nc"] = build(Cfg())
    return _CACHED["nc"]


def kernel(query, key, value, mask, Wq, bq, Wk, bk, Wv, bv, Wo, bo):
    cfg = Cfg()
    nc = get_built()
    in_maps = host_prep(query, key, value, mask, Wq, bq, Wk, bk, Wv, bv, Wo, bo, cfg)
    res = run_bass_kernel_spmd(nc, in_maps, core_ids=list(range(N_CORES)))
    B, S, DM = query.shape
    out = np.empty((B, S, DM), np.float32)
    for c in range(N_CORES):
        b, h = divmod(c, 2)
        out[b, h * cfg.SQ:(h + 1) * cfg.SQ, :] = res.results[c]["out"]
    return out


# revision 30
# speedup vs baseline: 1.3030x; 1.3030x over previous
"""MultiHeadedAttention Trainium2 kernel (8-core SPMD, data-parallel).

Sharding: 8 cores = (batch b in 0..3) x (query half in 0..1). Each core
computes out[b, half*1024:(half+1)*1024, :] independently - no collectives.

Per-core dataflow v2 (no DRAM staging; transposes on the PE):
  - x blocks [128, dm] fp32 DMA'd natural, cast fp32->bf16 on DVE,
    transposed 128x128 at a time on the PE (identity matmul) into bf16
    PSUM, evicted by ScalarE into xT layouts (contraction dim on
    partitions)
  - v: per-block xvT ring -> v natural [Sk, d] + ones column (Z trick)
  - mask: int32 blocks cast to bf16, PE-transposed, GpSimd-evicted to
    maskT
  - k/q: full xkT/xqT, projections tiled ns-outer/hp-inner so the
    PE consumes transposed blocks as they land; bias applied during
    PSUM eviction on DVE (tensor_scalar_add)
  - attention per head-pair: scores^T = kT.T @ qT (row-tiled head
    pairs); exp on ScalarE (scale 1/8 folded into Wq/bq); mask applied
    as bf16 multiply on DVE; PV psum rows 0..63 = sum_j v^T p, row 64
    = Z; finalize: reciprocal on DVE, partition-broadcast on GpSimd,
    multiply, SBUF->SBUF DMA hop into head-pair layout xattnT
  - out = xattnT.T @ WoT + R where R = bo + bv@WoT (PE-broadcast), fp32
"""
import numpy as np
import ml_dtypes

import concourse.bass as bass
import concourse.mybir as mybir
import concourse.tile as tile
from concourse import bacc
from concourse.bass_utils import run_bass_kernel_spmd
from concourse.masks import make_identity

F32 = mybir.dt.float32
BF16 = mybir.dt.bfloat16
I32 = mybir.dt.int32
AF = mybir.ActivationFunctionType
ALU = mybir.AluOpType

N_CORES = 8
DK = 64


def slices(total, chunk):
    return [(s, min(chunk, total - s)) for s in range(0, total, chunk)]


class Cfg:
    def __init__(self, SQ=1024, SK=2048, DM=1024, H=16, max_stage=5,
                 variant=0):
        self.variant = variant
        assert DM % 128 == 0 and SK % 128 == 0 and SQ % 128 == 0 and H % 2 == 0
        self.SQ, self.SK, self.DM, self.H = SQ, SK, DM, H
        self.KT = DM // 128          # dm contraction chunks
        self.HP = H // 2             # head pairs
        self.NJ = SK // 128          # Sk tiles
        self.SQS = min(1024, SQ)     # attention Sq slice width (2 psum banks)
        self.max_stage = max_stage   # debug: truncate kernel after stage N
        assert SQ % self.SQS == 0
        assert H * DK == DM


def emit_kernel(tc, cfg, io):
    nc = tc.nc
    C = cfg
    xq, xk, xv, msk = io["xq"], io["xk"], io["xv"], io["mask"]
    w_dram = {"q": io["wqt"], "k": io["wkt"], "v": io["wvt"], "o": io["wot"]}
    bql, bkl, rfull = io["bql"], io["bkl"], io["rfull"]
    out = io["out"]

    pools = {}

    def open_pool(name, bufs=1, space="SBUF", side=None):
        pools[name] = tc.alloc_tile_pool(name=name, bufs=bufs, space=space,
                                         side=side)
        return pools[name]

    def release_pool(name):
        pools[name].release()
        del pools[name]

    persist = open_pool("persist", 1)
    # Stage A-C PSUM: proj psums (tag s, 2x2 banks) + transpose staging
    # (tag t, 2x1 bank). Released before attention, which re-opens the
    # arena as scores (2x2) + PV accumulators (2x2).
    ps_s = open_pool("ps_s", 2, space="PSUM")
    ps_t = open_pool("ps_t", 3, space="PSUM")
    # wo on the right-side SBUF stack: loaded late, lives to the end
    wo_pool = open_pool("wo", 1, side="right")
    ring = open_pool("ring", 1)      # xn/xb: lives through stages A-C
    ringA = open_pool("ringA", 1)    # xvt/mi/mb: stage A only

    # ---------------- persistent tiles ----------------
    qT_sb = persist.tile([128, C.HP * C.SQ], BF16, name="qT_sb")
    kT_sb = persist.tile([128, C.HP * C.SK], BF16, name="kT_sb")
    v_sb = persist.tile([128, C.NJ * C.H * 65], BF16, name="v_sb")
    xattnT_sb = persist.tile([128, C.HP * C.SQ], BF16, name="xattnT_sb")
    maskT_sb = persist.tile([128, C.NJ * C.SQ], BF16, name="maskT_sb")
    R_sb = persist.tile([128, C.DM], F32, name="R_sb")
    bql_sb = persist.tile([128, C.HP], F32, name="bql_sb")
    bkl_sb = persist.tile([128, C.HP], F32, name="bkl_sb")
    ident_sb = persist.tile([128, 128], BF16, name="ident_sb")

    make_identity(nc, ident_sb[:])
    nc.sync.dma_start(bql_sb[:], bql[:])
    nc.sync.dma_start(bkl_sb[:], bkl[:])

    PS_F = max(C.SQS, 512)

    maskTv = maskT_sb.rearrange("p (j s) -> p j s", j=C.NJ)

    def load_w(pool, which, name):
        w_sb = pool.tile([128, C.KT * C.DM], BF16, name=name)
        for kt in range(C.KT):
            nc.sync.dma_start(w_sb[:, kt * C.DM:(kt + 1) * C.DM],
                               w_dram[which][kt * 128:(kt + 1) * 128, :])
        return w_sb

    def load_x_block(x_in, r):
        """DMA 128-row block r of x (fp32) and cast to bf16; returns the
        bf16 tile (natural layout)."""
        xn = ring.tile([128, C.DM], F32, name="xn", tag="xn", bufs=3)
        nc.sync.dma_start(xn[:], x_in[r * 128:(r + 1) * 128, :])
        xb = ring.tile([128, C.DM], BF16, name="xb", tag="xb", bufs=5)
        nc.vector.tensor_copy(xb[:], xn[:])
        return xb

    def tev_block(xb, dst):
        """PE-transpose a bf16 natural block into dst [p, kt, 128]."""
        pt = ps_t.tile([128, C.KT * 128], BF16, name="pt", tag="t",
                       padded_shape=[128, 1024])
        for kt in range(C.KT):
            nc.tensor.transpose(pt[:, kt * 128:(kt + 1) * 128],
                                xb[:, kt * 128:(kt + 1) * 128], ident_sb[:])
        nc.scalar.activation(
            dst,
            pt.rearrange("p (kt s) -> p kt s", kt=C.KT),
            AF.Copy,
        )

    def emit_x_block(x_in, r, dst):
        tev_block(load_x_block(x_in, r), dst)

    def emit_mask_block(r):
        """Load 128-row block r of mask (int32), cast to bf16, PE-transpose,
        GpSimd-evict into maskT."""
        for half in range(C.SK // 1024):
            mi = ringA.tile([128, 1024], I32, name="mi", tag="mi", bufs=2)
            nc.gpsimd.dma_start(
                mi[:], msk[r * 128:(r + 1) * 128,
                           half * 1024:(half + 1) * 1024])
            mb = ringA.tile([128, 1024], BF16, name="mb", tag="mb", bufs=2)
            nc.gpsimd.tensor_copy(mb[:], mi[:])
            pt = ps_t.tile([128, 1024], BF16, name="ptm", tag="t",
                           padded_shape=[128, 1024])
            for c in range(8):
                nc.tensor.transpose(pt[:, c * 128:(c + 1) * 128],
                                    mb[:, c * 128:(c + 1) * 128], ident_sb[:])
            nc.scalar.activation(
                maskTv[:, half * 8:(half + 1) * 8, r * 128:(r + 1) * 128],
                pt.rearrange("p (c s) -> p c s", c=8),
                AF.Copy,
            )

    # ---------------- stage A: v (ring) + mask, interleaved ----------------
    # first x-block loads go out before the wv weight-load issues so the
    # PE's first transposes aren't queued behind 2MB of weight DMA
    wv_pool = open_pool("wv", 1)
    pre_v = [load_x_block(xv, r) for r in range(3)]
    wv_sb = load_w(wv_pool, "v", "w_v")
    v_view = v_sb.rearrange("p (j h c) -> p j h c", j=C.NJ, c=65)

    NBV = C.SK // 128
    xvt_blocks = [None] * NBV

    def emit_v_block(r):
        xvt = ringA.tile([128, C.KT * 128], BF16, name="xvt", tag="xvt", bufs=3)
        xvt_blocks[r] = xvt
        xb = pre_v[r] if r < len(pre_v) else load_x_block(xv, r)
        tev_block(xb, xvt.rearrange("p (kt s) -> p kt s", kt=C.KT)[:, :, :])

    def emit_v_proj(j):
        xvt = xvt_blocks[j]
        for (ds_, dw) in slices(C.DM, 512):
            hs = ds_ // DK
            ps = ps_s.tile([128, dw], F32, name="ps_v", tag="s",
                           padded_shape=[128, PS_F])
            for kt in range(C.KT):
                nc.tensor.matmul(
                    ps[:],
                    xvt[:, kt * 128:(kt + 1) * 128],
                    wv_sb[:, kt * C.DM + ds_: kt * C.DM + ds_ + dw],
                    start=(kt == 0), stop=(kt == C.KT - 1),
                )
            nc.vector.tensor_copy(
                v_view[:, j, hs:hs + dw // DK, 0:64],
                ps.rearrange("p (h c) -> p h c", c=DK),
            )
        xvt_blocks[j] = None

    # interleave: 3-block lead for the transpose pipeline; one mask block
    # every other v block
    LEAD = 3
    n_mask = C.SQ // 128
    mq = list(range(n_mask))
    for r in range(LEAD):
        emit_v_block(r)
    pre_k = None
    for r in range(NBV):
        if r + LEAD < NBV:
            emit_v_block(r + LEAD)
        if r == NBV - 2:
            pre_k = [load_x_block(xk, rr) for rr in range(4)]
        emit_v_proj(r)
        if r >= 2 and mq:
            emit_mask_block(mq.pop(0))
    for r in mq:
        emit_mask_block(r)
    nc.vector.memset(v_view[:, :, :, 64:65], 1.0)
    release_pool("wv")
    release_pool("ringA")

    if C.max_stage <= 2:
        for pl in reversed(list(pools.values())):
            pl.release()
        return

    # ---------------- stage B/C: k then q projections ----------------
    # ns-outer / hp-inner: output columns [ns, ns+512) need only the four
    # x row-blocks ns/128.., held in a 2-deep ring of transposed groups.
    def proj_stage(pool, x_in, w_sb, T_sb, S, bias_sb, gtag, pre_xbs=None,
                   preload_next=None):
        chunks = slices(S, 512)
        preloaded = None

        def emit_group(ci, xbs=None):
            grp = pool.tile([128, C.KT * 512], BF16, name=f"{gtag}{ci}",
                            tag=gtag, bufs=2)
            gv = grp.rearrange("p (kt s) -> p kt s", kt=C.KT)
            for rr in range(4):
                xb = xbs[rr] if xbs else load_x_block(x_in, 4 * ci + rr)
                tev_block(xb, gv[:, :, rr * 128:(rr + 1) * 128])
            return grp

        nxt = emit_group(0, pre_xbs)
        for ci, (ns, nw) in enumerate(chunks):
            grp, nxt = nxt, (emit_group(ci + 1) if ci + 1 < len(chunks) else None)
            if ci == len(chunks) - 1 and preload_next is not None:
                preloaded = preload_next()
            for hp in range(C.HP):
                ps = ps_s.tile([128, nw], F32, name="ps_p", tag="s",
                               padded_shape=[128, PS_F])
                for kt in range(C.KT):
                    nc.tensor.matmul(
                        ps[:],
                        w_sb[:, kt * C.DM + hp * 128: kt * C.DM + (hp + 1) * 128],
                        grp[:, kt * 512: kt * 512 + nw],
                        start=(kt == 0), stop=(kt == C.KT - 1),
                    )
                nc.vector.tensor_scalar_add(
                    T_sb[:, hp * S + ns: hp * S + ns + nw],
                    ps[:], bias_sb[:, hp:hp + 1],
                )
        return preloaded

    kx = open_pool("kx", 1)
    wk_sb = load_w(kx, "k", "w_k")
    pre_q = proj_stage(kx, xk, wk_sb, kT_sb, C.SK, bkl_sb, "xkg",
                       pre_xbs=pre_k,
                       preload_next=lambda: [load_x_block(xq, r)
                                             for r in range(4)])
    release_pool("kx")

    qx = open_pool("qx", 1)
    wq_sb = load_w(qx, "q", "w_q")
    wo_sb = wo_pool.tile([128, C.KT * C.DM], BF16, name="wo_sb")
    for kt in range(C.KT):
        nc.sync.dma_start(wo_sb[:, kt * C.DM:(kt + 1) * C.DM],
                          w_dram["o"][kt * 128:(kt + 1) * 128, :])
    proj_stage(qx, xq, wq_sb, qT_sb, C.SQ, bql_sb, "xqg", pre_xbs=pre_q)

    release_pool("qx")
    release_pool("ring")
    release_pool("ps_t")
    release_pool("ps_s")

    if C.max_stage <= 3:
        for pl in reversed(list(pools.values())):
            pl.release()
        return

    # ---------------- attention ----------------
    nc.sync.dma_start(R_sb[:], rfull[:])
    ps_sc = open_pool("ps_sc", 2, space="PSUM")
    ps_pv = open_pool("ps_pv", 2, space="PSUM")
    attn = open_pool("attn", 1)

    # One continuous software-pipelined stream over (hp, j): the PV matmuls
    # lag the scores/exp/mask chain by PIPE items, and the lag crosses
    # head-pair boundaries, so the drain of head-pair hp overlaps the
    # scores/exp of head-pair hp+1 and ScalarE never idles at boundaries.
    sq, sw = 0, C.SQS
    assert C.SQS == C.SQ
    PIPE = 4
    pv_of = {}
    pm_hist = []

    def emit_pv(hp, jj, pms):
        if jj == 0:
            pv_of[hp] = [
                ps_pv.tile([65, sw], F32, name=f"ps_pv{i}", tag="pv",
                           padded_shape=[128, PS_F])
                for i in range(2)
            ]
        pv = pv_of[hp]
        for i in range(2):
            for (qs, qw) in slices(sw, 512):
                nc.tensor.matmul(
                    pv[i][:, qs:qs + qw], v_view[:, jj, 2 * hp + i, :],
                    pms[i][:, qs:qs + qw],
                    start=(jj == 0), stop=(jj == C.NJ - 1),
                )

    def emit_finalize(hp):
        pv = pv_of.pop(hp)
        for i in range(2):
            # Z row to SBUF (the custom DVE reciprocal cannot source PSUM
            # on hardware), reciprocal on DVE, partition-broadcast on
            # GpSimd, normalize on DVE straight from the PSUM accumulator
            zrow = attn.tile([1, sw], F32, name="zrow", tag="zrow", bufs=2,
                             padded_shape=[1, C.SQS])
            nc.vector.tensor_copy(zrow[:], pv[i][64:65, :])
            zr1 = attn.tile([1, sw], F32, name="zr1", tag="zr1", bufs=2,
                            padded_shape=[1, C.SQS])
            nc.vector.reciprocal_approx_fast(out=zr1[:], in_=zrow[:])
            zrb = attn.tile([64, sw], F32, name="zrb", tag="zrb", bufs=2,
                            padded_shape=[64, C.SQS])
            nc.gpsimd.partition_broadcast(zrb[:], zr1[:])
            tmp = attn.tile([64, sw], BF16, name="xat_t", tag="xat_t",
                            bufs=2, padded_shape=[64, C.SQS])
            nc.vector.tensor_tensor(out=tmp[:], in0=pv[i][0:64, :],
                                    in1=zrb[:], op=ALU.mult)
            # partition hop: rows 0..63 -> xattnT pair rows 64i..64i+64
            nc.sync.dma_start(
                xattnT_sb[64 * i:64 * (i + 1),
                          hp * C.SQ + sq: hp * C.SQ + sq + sw],
                tmp[:],
            )

    def drain_one():
        (dhp, jj), pp = pm_hist.pop(0)
        emit_pv(dhp, jj, pp)
        if jj == C.NJ - 1:
            emit_finalize(dhp)

    for hp in range(C.HP):
        for j in range(C.NJ):
            # emit the pipelined PV first so the PE has ready work queued
            # ahead of the ss-slot wait of this iteration's scores
            if len(pm_hist) >= PIPE:
                drain_one()
            pms = []
            sss = [ps_sc.tile([128, sw], F32, name=f"ps_sc{i}", tag="sc",
                              padded_shape=[128, PS_F]) for i in range(2)]
            # interleave the two heads' MMs so the row-tiled (0,0)/(64,0)
            # pairs sit adjacent in the PE queue and run concurrently
            for (qs, qw) in slices(sw, 512):
                for i in range(2):
                    nc.tensor.matmul(
                        sss[i][:, qs:qs + qw],
                        kT_sb[i * 64:(i + 1) * 64,
                              hp * C.SK + j * 128: hp * C.SK + (j + 1) * 128],
                        qT_sb[i * 64:(i + 1) * 64,
                              hp * C.SQ + sq + qs: hp * C.SQ + sq + qs + qw],
                        start=True, stop=True,
                    )
            for i in range(2):
                pe = attn.tile([128, sw], BF16, name="p_exp", tag="pexp",
                               bufs=4, padded_shape=[128, C.SQS])
                nc.scalar.activation(pe[:], sss[i][:], AF.Exp)
                pm = attn.tile([128, sw], BF16, name="p_msk", tag="pmask",
                               bufs=10, padded_shape=[128, C.SQS])
                mask_eng = (nc.gpsimd if (C.variant == 1 and hp > 0 and j < 2)
                            else nc.vector)
                mask_eng.tensor_tensor(
                    out=pm[:], in0=pe[:],
                    in1=maskT_sb[:, j * C.SQ + sq: j * C.SQ + sq + sw],
                    op=ALU.mult,
                )
                pms.append(pm)
            pm_hist.append(((hp, j), pms))
    while pm_hist:
        drain_one()

    if C.max_stage <= 4:
        for pl in reversed(list(pools.values())):
            pl.release()
        return

    # ---------------- epilogue: output projection ----------------
    # hp-outer in groups of four row-blocks (reusing the attention PSUM
    # pools: 2 score slots + 2 pv slots = 4 concurrent accumulators), so
    # only the final head-pair's matmuls wait on the attention finalize.
    release_pool("attn")
    epi = open_pool("epi", 1)

    NM = C.SQ // 128
    for (ns, nw) in slices(C.DM, 512):
        for mg in range(0, NM, 4):
            po = [ps_sc.tile([128, nw], F32, name=f"ps_o{m}", tag="sc",
                             padded_shape=[128, PS_F]) for m in range(2)]
            po += [ps_pv.tile([128, nw], F32, name=f"ps_o{m+2}", tag="pv",
                              padded_shape=[128, PS_F]) for m in range(2)]
            for hp in range(C.HP):
                for mi_ in range(4):
                    m = mg + mi_
                    nc.tensor.matmul(
                        po[mi_][:],
                        xattnT_sb[:, hp * C.SQ + m * 128: hp * C.SQ + (m + 1) * 128],
                        wo_sb[:, hp * C.DM + ns: hp * C.DM + ns + nw],
                        start=(hp == 0), stop=(hp == C.HP - 1),
                    )
            for mi_ in range(4):
                m = mg + mi_
                ot = epi.tile([128, nw], F32, name="out_sb", tag="out_sb",
                              bufs=4, padded_shape=[128, 512])
                nc.vector.tensor_tensor(out=ot[:], in0=po[mi_][:],
                                        in1=R_sb[:, ns:ns + nw], op=ALU.add)
                nc.sync.dma_start(out[m * 128:(m + 1) * 128, ns:ns + nw], ot[:])

    for pl in reversed(list(pools.values())):
        pl.release()


def build(cfg, reps=1):
    nc = bacc.Bacc("TRN2", target_bir_lowering=False, debug=False)
    C = cfg
    io = {
        "xq": nc.dram_tensor("xq", [C.SQ, C.DM], F32, kind="ExternalInput").ap(),
        "xk": nc.dram_tensor("xk", [C.SK, C.DM], F32, kind="ExternalInput").ap(),
        "xv": nc.dram_tensor("xv", [C.SK, C.DM], F32, kind="ExternalInput").ap(),
        "mask": nc.dram_tensor("mask", [C.SQ, C.SK], I32, kind="ExternalInput").ap(),
        "wqt": nc.dram_tensor("wqt", [C.DM, C.DM], BF16, kind="ExternalInput").ap(),
        "wkt": nc.dram_tensor("wkt", [C.DM, C.DM], BF16, kind="ExternalInput").ap(),
        "wvt": nc.dram_tensor("wvt", [C.DM, C.DM], BF16, kind="ExternalInput").ap(),
        "wot": nc.dram_tensor("wot", [C.DM, C.DM], BF16, kind="ExternalInput").ap(),
        "bql": nc.dram_tensor("bql", [128, C.HP], F32, kind="ExternalInput").ap(),
        "bkl": nc.dram_tensor("bkl", [128, C.HP], F32, kind="ExternalInput").ap(),
        "rfull": nc.dram_tensor("rfull", [128, C.DM], F32, kind="ExternalInput").ap(),
        "out": nc.dram_tensor("out", [C.SQ, C.DM], F32, kind="ExternalOutput").ap(),
    }
    with tile.TileContext(nc) as tc:
        for _ in range(reps):
            emit_kernel(tc, cfg, io)
    nc.compile()
    return nc


def host_prep(query, key, value, mask, Wq, bq, Wk, bk, Wv, bv, Wo, bo, cfg):
    """Host-side layout prep (weight transpose/cast, per-core slicing)."""
    C = cfg
    bf = ml_dtypes.bfloat16
    wqt = np.ascontiguousarray((Wq.T * 0.125).astype(bf))   # 1/sqrt(dk) folded
    wkt = np.ascontiguousarray(Wk.T.astype(bf))
    wvt = np.ascontiguousarray(Wv.T.astype(bf))
    wot = np.ascontiguousarray(Wo.T.astype(bf))
    bql = np.ascontiguousarray((bq * 0.125).reshape(C.HP, 128).T.astype(np.float32))
    bkl = np.ascontiguousarray(bk.reshape(C.HP, 128).T.astype(np.float32))
    r_row = (bv.astype(np.float32) @ Wo.T.astype(np.float32)
             + bo.astype(np.float32))
    rfull = np.ascontiguousarray(
        np.broadcast_to(r_row, (128, C.DM)).astype(np.float32))
    shared = dict(wqt=wqt, wkt=wkt, wvt=wvt, wot=wot, bql=bql, bkl=bkl,
                  rfull=rfull)
    in_maps = []
    B = query.shape[0]
    halves = query.shape[1] // C.SQ
    for c in range(B * halves):
        b, h = divmod(c, halves)
        m = dict(shared)
        m["xq"] = np.ascontiguousarray(query[b, h * C.SQ:(h + 1) * C.SQ, :])
        m["xk"] = np.ascontiguousarray(key[b])
        m["xv"] = np.ascontiguousarray(value[b])
        m["mask"] = np.ascontiguousarray(mask[b, h * C.SQ:(h + 1) * C.SQ, :])
        in_maps.append(m)
    return in_maps


_CACHED = {}


def get_built():
    if "nc" not in _CACHED:
        _CACHED["nc"] = build(Cfg())
    return _CACHED["nc"]


def kernel(query, key, value, mask, Wq, bq, Wk, bk, Wv, bv, Wo, bo):
    cfg = Cfg()
    nc = get_built()
    in_maps = host_prep(query, key, value, mask, Wq, bq, Wk, bk, Wv, bv, Wo, bo, cfg)
    res = run_bass_kernel_spmd(nc, in_maps, core_ids=list(range(N_CORES)))
    B, S, DM = query.shape
    out = np.empty((B, S, DM), np.float32)
    for c in range(N_CORES):
        b, h = divmod(c, 2)
        out[b, h * cfg.SQ:(h + 1) * cfg.SQ, :] = res.results[c]["out"]
    return out
